# revision 13
# baseline (speedup 1.0000x reference)
"""Multi-head causal attention (B=2, S=2048, D=1024, H=16) on 8 trn2 cores.

Sharding: core c -> batch b = c//4, head-group g = c%4 (4 heads each).
Tensor-parallel on Wq/Wk/Wv (column) and Wo (row); the Wo all-reduce is the
host-side sum of the 4 per-core partials of each batch.

Device layout notes:
 - All activations are kept "transposed" (feature dim on partitions) so every
   matmul contraction has its axis on partitions with zero on-chip transposes.
 - Scores are built as S.T (keys on partitions, queries free): softmax sums
   become a PE matmul against an appended ones-column of V, and the second
   attention matmul needs V in natural layout (which the projection emits
   directly).
 - No row-max subtraction: scores are ~N(0, 0.4) after the 1/8 scale, exp is
   safe in fp32 by a huge margin.
 - Matmuls run as float32r (full-rate fp32 PE mode, ~2e-4 rel err), PSUM fp32.
 - Causal mask applied as a post-exp multiply on diagonal blocks only;
   strictly-upper blocks are skipped entirely (compute and exp savings).
"""

import numpy as np

import concourse.bacc as bacc
import concourse.mybir as mybir
import concourse.tile as tile
from concourse.bass_utils import run_bass_kernel_spmd

B, S, D, H = 2, 2048, 1024, 16
DK = 64            # head dim
HG = 4             # heads per core
GD = HG * DK       # 256 dims per head-group
P = 128
NQ = 512           # query chunk (free dim of score blocks)
QB = S // NQ       # 4 query superblocks
KB = S // P        # 16 key blocks
KO = D // P        # 8 contraction tiles for the projections
F32 = mybir.dt.float32
I32 = mybir.dt.int32
F32R = mybir.dt.float32r


def _r(ap):
    return ap


def build(mode):
    assert mode in ("tril", "ones", "general")
    nc = bacc.Bacc(None, target_bir_lowering=False)

    xqT = nc.dram_tensor("xqT", [D, S], F32R, kind="ExternalInput")
    xkT = nc.dram_tensor("xkT", [D, S], F32R, kind="ExternalInput")
    xvT = nc.dram_tensor("xvT", [D, S], F32R, kind="ExternalInput")
    wqT = nc.dram_tensor("wqT", [D, GD], F32R, kind="ExternalInput")
    wkT = nc.dram_tensor("wkT", [D, GD], F32R, kind="ExternalInput")
    wvT = nc.dram_tensor("wvT", [D, GD], F32R, kind="ExternalInput")
    woT = nc.dram_tensor("woT", [GD, D], F32R, kind="ExternalInput")
    maskd = maskT = None
    if mode == "tril":
        maskd = nc.dram_tensor("maskd", [4, P, NQ], I32, kind="ExternalInput")
    elif mode == "general":
        maskT = nc.dram_tensor("maskT", [S, S], I32, kind="ExternalInput")
    outT = nc.dram_tensor("outT", [D, S], F32, kind="ExternalOutput")

    with tile.TileContext(nc) as tc:
        with (
            tc.tile_pool(name="wpool", bufs=1) as wpool,
            tc.tile_pool(name="perm", bufs=1) as perm,
            tc.tile_pool(name="xs", bufs=3) as xsp,
            tc.tile_pool(name="es", bufs=6) as esp,
            tc.tile_pool(name="ob", bufs=2) as obp,
            tc.tile_pool(name="outp", bufs=4) as outp,
            tc.tile_pool(name="small", bufs=3) as smallp,
            tc.tile_pool(name="gmask", bufs=2) as gmp,
            tc.tile_pool(name="psS", bufs=2, space="PSUM") as psS,
            tc.tile_pool(name="psB", bufs=2, space="PSUM") as psB,
            tc.tile_pool(name="psO", bufs=2, space="PSUM") as psO,
        ):
            # ---- persistent weights ----
            wq_sb = wpool.tile([P, KO, GD], F32R, tag="wq")
            wk_sb = wpool.tile([P, KO, GD], F32R, tag="wk")
            wv_sb = wpool.tile([P, KO, GD], F32R, tag="wv")
            wo_sb = wpool.tile([P, 2, D], F32R, tag="wo")
            nc.sync.dma_start(wq_sb, wqT.rearrange("(ko p) m -> p ko m", p=P))
            nc.scalar.dma_start(wk_sb, wkT.rearrange("(ko p) m -> p ko m", p=P))

            vcol = wpool.tile([P, 1], F32, tag="vcol")
            nc.vector.memset(vcol, 1.0)

            maskf = None
            if mode == "tril":
                maski = wpool.tile([P, 4, NQ], I32, tag="maski")
                nc.scalar.dma_start(maski, maskd.rearrange("k p q -> p k q"))
                maskf = wpool.tile([P, 4, NQ], F32R, tag="maskf")
                nc.gpsimd.tensor_copy(out=maskf, in_=maski)

            # ---- persistent projection outputs ----
            qT_sb = [perm.tile([P, S], F32R, tag=f"qT{i}", name=f"qT{i}") for i in range(2)]
            kT_sb = [perm.tile([P, S], F32R, tag=f"kT{i}", name=f"kT{i}") for i in range(2)]
            v_sb = [perm.tile([P, HG, DK + 1], F32R, tag=f"v{i}", name=f"v{i}")
                    for i in range(KB)]

            eps_ap = wpool.tile([1, 1], F32, tag="eps")
            nc.vector.memset(eps_ap, 1e-30)

            xqTr = xqT.rearrange("(ko p) s -> p ko s", p=P)
            xkTr = xkT.rearrange("(ko p) s -> p ko s", p=P)
            xvTr = xvT.rearrange("(ko p) s -> p ko s", p=P)

            def load_chunk(c):
                # stream x.T slices for sequence chunk c (q, k, v);
                # k goes via the ACT HWDGE queue to parallelize issue.
                # Per-ko DMAs keep every transfer small so downstream queue
                # semaphores clear early and subtile deps unlock sooner.
                tiles = []
                for xTr, eng in ((xqTr, nc.sync), (xkTr, nc.scalar),
                                 (xvTr, nc.sync)):
                    xs = xsp.tile([P, KO, NQ], F32R, tag="xs", name="xs")
                    for ko in range(KO):
                        eng.dma_start(
                            xs[:, ko, :], xTr[:, ko, c * NQ:(c + 1) * NQ])
                    tiles.append(xs)
                return tiles

            def proj_qk_chunk(c, xst):
                # q/k projections for sequence chunk c -> qT/kT columns [c*NQ, (c+1)*NQ)
                for xs, w_sb, dst in ((xst[0], wq_sb, qT_sb), (xst[1], wk_sb, kT_sb)):
                    for hp in range(2):
                        ps = psB.tile([P, NQ], F32, tag="mm1", name="ps_qk")
                        for ko in range(KO):
                            nc.tensor.matmul(
                                ps[:, :],
                                w_sb[:, ko, hp * P:(hp + 1) * P],
                                xs[:, ko, :],
                                start=(ko == 0), stop=(ko == KO - 1),
                            )
                        nc.vector.tensor_copy(
                            out=dst[hp][:, c * NQ:(c + 1) * NQ], in_=ps[:, :])

            def proj_v_chunk(c, xst):
                # v projection for sequence chunk c -> v_sb[4c .. 4c+3]
                xs = xst[2]
                for si in range(4):
                    sq = 4 * c + si
                    ps = psB.tile([P, NQ], F32, tag="mm1", name="ps_v")
                    for ko in range(KO):
                        nc.tensor.matmul(
                            ps[:, :GD],
                            xs[:, ko, si * P:(si + 1) * P],
                            wv_sb[:, ko, :],
                            start=(ko == 0), stop=(ko == KO - 1),
                        )
                    nc.vector.tensor_copy(
                        out=v_sb[sq][:, :, 0:DK],
                        in_=ps[:, :GD].rearrange("p (h d) -> p h d", h=HG))
                    nc.gpsimd.tensor_copy(
                        out=v_sb[sq][:, :, DK:DK + 1],
                        in_=vcol[:, None, :].to_broadcast((P, HG, 1)))

            def attention_qb(qb):
                nkb = 4 * (qb + 1) if mode == "tril" else KB

                mgf = None
                if mode == "general":
                    mgf = gmp.tile([P, KB, NQ], F32R, tag="mgf", name="mgf")
                    nc.sync.dma_start(
                        mgf.bitcast(I32),
                        maskT.rearrange("(kb p) q -> p kb q", p=P)[
                            :, :, qb * NQ:(qb + 1) * NQ])
                    nc.vector.tensor_copy(out=mgf, in_=mgf.bitcast(I32))

                O_sb = [obp.tile([P, NQ], F32R, tag=f"O{i}", name=f"O{i}")
                        for i in range(2)]
                for hp in range(2):
                    pso = [psO.tile([DK + 1, NQ], F32, tag="O", name=f"pso{hh}")
                           for hh in range(2)]
                    for kp in range(nkb // 2):
                        sp2 = [psS.tile([P, 2, NQ], F32, tag="mm2", name=f"sp{hh}")
                               for hh in range(2)]
                        es2 = [esp.tile([P, 2, NQ], F32R, tag="es", name=f"es{hh}")
                               for hh in range(2)]
                        # both heads' score matmuls adjacent: disjoint PE row
                        # groups (K=64 at base partitions 0 / 64) run overlapped
                        for j in (0, 1):
                            kb = 2 * kp + j
                            for hh in range(2):
                                nc.tensor.matmul(
                                    sp2[hh][:, j, :],
                                    kT_sb[hp][hh * DK:(hh + 1) * DK,
                                              kb * P:(kb + 1) * P],
                                    qT_sb[hp][hh * DK:(hh + 1) * DK,
                                              qb * NQ:(qb + 1) * NQ],
                                    start=True, stop=True,
                                )
                        for hh in range(2):
                            h = 2 * hp + hh
                            nc.scalar.activation(
                                out=es2[hh][:], in_=sp2[hh][:],
                                func=mybir.ActivationFunctionType.Exp, scale=0.125)
                            for j in (0, 1):
                                kb = 2 * kp + j
                                if mode == "tril" and kb >= 4 * qb:
                                    nc.vector.tensor_mul(
                                        out=es2[hh][:, j, :], in0=es2[hh][:, j, :],
                                        in1=maskf[:, kb - 4 * qb, :])
                                elif mode == "general":
                                    nc.vector.tensor_mul(
                                        out=es2[hh][:, j, :], in0=es2[hh][:, j, :],
                                        in1=mgf[:, kb, :])
                            for j in (0, 1):
                                kb = 2 * kp + j
                                nc.tensor.matmul(
                                    pso[hh][:, :],
                                    v_sb[kb][:, h, :],
                                    es2[hh][:, j, :],
                                    start=(kb == 0), stop=(kb == nkb - 1),
                                )
                    for hh in range(2):
                        # normalize: O = O_unnorm * 1/sum, sum broadcast over
                        # partitions via a K=1 PE matmul, reciprocal on DVE
                        sum_sb = smallp.tile([1, NQ], F32, tag="sum", name="sum_sb")
                        nc.scalar.activation(
                            out=sum_sb, in_=pso[hh][DK:DK + 1, :],
                            func=mybir.ActivationFunctionType.Identity,
                            bias=eps_ap, scale=1.0)
                        recip_sb = smallp.tile([1, NQ], F32, tag="recip", name="recip_sb")
                        nc.vector.reciprocal_approx_fast(
                            out=recip_sb, in_=sum_sb)
                        bc_sb = smallp.tile([DK, NQ], F32, tag="bc", name="bc_sb")
                        nc.gpsimd.partition_broadcast(bc_sb, recip_sb)
                        nc.vector.tensor_mul(
                            out=O_sb[hp][hh * DK:(hh + 1) * DK, :],
                            in0=pso[hh][0:DK, :], in1=bc_sb)

                return O_sb

            def outproj_qb(qb, O_sb):
                for od in range(KO):
                    po = psB.tile([P, NQ], F32, tag="mm1", name="po")
                    for t in range(2):
                        nc.tensor.matmul(
                            po[:, :],
                            wo_sb[:, t, od * P:(od + 1) * P],
                            O_sb[t],
                            start=(t == 0), stop=(t == 1),
                        )
                    osb = outp.tile([P, NQ], F32, tag="osb", name="osb")
                    nc.vector.tensor_copy(out=osb, in_=po[:, :])
                    nc.sync.dma_start(
                        outT[od * P:(od + 1) * P, qb * NQ:(qb + 1) * NQ], osb)

            # chunk-interleaved schedule: causal attention for superblock c
            # needs only k/v chunks <= c, so projections and attention overlap
            nc.scalar.dma_start(wv_sb, wvT.rearrange("(ko p) m -> p ko m", p=P))
            xst = load_chunk(0)
            proj_qk_chunk(0, xst)
            proj_v_chunk(0, xst)
            nc.sync.dma_start(wo_sb, woT.rearrange("(t p) m -> p t m", p=P))
            for c in range(QB):
                O_sb = attention_qb(c)
                if c + 1 < QB:
                    # next chunk's projections fill the PE while this
                    # superblock's softmax tails complete
                    xst = load_chunk(c + 1)
                    proj_qk_chunk(c + 1, xst)
                    proj_v_chunk(c + 1, xst)
                outproj_qb(c, O_sb)

    nc.compile()
    return nc


_CACHE = {}


def _get(mode):
    if mode not in _CACHE:
        _CACHE[mode] = build(mode)
    return _CACHE[mode]


def kernel(Q, K, V, Wq, Wk, Wv, Wo, mask, _want_results=False):
    Q = np.asarray(Q, dtype=np.float32)
    K = np.asarray(K, dtype=np.float32)
    V = np.asarray(V, dtype=np.float32)
    Wq = np.asarray(Wq, dtype=np.float32)
    Wk = np.asarray(Wk, dtype=np.float32)
    Wv = np.asarray(Wv, dtype=np.float32)
    Wo = np.asarray(Wo, dtype=np.float32)
    m2 = np.asarray(mask).reshape(S, S)

    if np.array_equal(m2, np.tril(np.ones((S, S), m2.dtype))):
        mode = "tril"
    elif np.all(m2 != 0):
        mode = "ones"
    else:
        mode = "general"

    nc = _get(mode)

    xT = {}
    for b in range(B):
        xT[("q", b)] = np.ascontiguousarray(Q[b].T)
        xT[("k", b)] = np.ascontiguousarray(K[b].T)
        xT[("v", b)] = np.ascontiguousarray(V[b].T)

    mT = None
    maskd = None
    if mode == "general":
        mT = np.ascontiguousarray(m2.T.astype(np.int32))
    elif mode == "tril":
        mTf = m2.T.astype(np.int32)
        maskd = np.ascontiguousarray(
            mTf[:NQ, :NQ].reshape(4, P, NQ))

    in_maps = []
    for c in range(8):
        b, g = divmod(c, 4)
        sl = slice(g * GD, (g + 1) * GD)
        im = {
            "xqT": xT[("q", b)],
            "xkT": xT[("k", b)],
            "xvT": xT[("v", b)],
            "wqT": np.ascontiguousarray(Wq[sl, :].T),
            "wkT": np.ascontiguousarray(Wk[sl, :].T),
            "wvT": np.ascontiguousarray(Wv[sl, :].T),
            "woT": np.ascontiguousarray(Wo[:, sl].T),
        }
        if mode == "tril":
            im["maskd"] = maskd
        elif mode == "general":
            im["maskT"] = mT
        in_maps.append(im)

    res = run_bass_kernel_spmd(nc, in_maps, core_ids=list(range(8)))

    out = np.empty((B, S, D), dtype=np.float32)
    for b in range(B):
        acc = res.results[4 * b]["outT"].copy()
        for g in range(1, 4):
            acc += res.results[4 * b + g]["outT"]
        out[b] = acc.T
    if _want_results:
        return out, res
    return out
